# revision 1
# baseline (speedup 1.0000x reference)
"""Trainium2 Bass kernel for windowed sigmoid-attention (nn_Attention_24927990186215).

Reference computation (per full input):
    x: [16, 16, 16, 16, 512]  (b, nh, nw, t, d) -- windows of T=16 tokens
    q/k/v = x @ W{q,k,v} + b{q,k,v}; split into H=8 heads of 64
    scores = q @ k^T / sqrt(64) within each 16-token window
    probs = sigmoid(scores)  (elementwise, NOT softmax)
    ctx = probs @ v;  out = ctx @ Wo + bo
Sharding: data-parallel over batch dim (16) across 8 cores -> 2 batches
(8192 tokens) per core.

Per-core dataflow (all matmuls on the PE):
  - the host pre-transposes x and ships three feature-major copies: bf16
    (for the v projection) and an fp8e4m3 hi/lo residual pair (x ~ xh + xl)
    for the q/k projections.  No on-device transposes are needed.
  - q^T/k^T are computed feature-major with fp8 DoubleRow matmuls (two
    128-row contraction tiles per pass, 2x PE throughput): three residual
    cross terms xh*Wh + xh*Wl + xl*Wh (the xl*Wl term is ~0.1% and is
    dropped), 6 DoubleRow matmuls per 512-wide chunk vs 4 bf16 matmuls --
    25% fewer PE cycles at ~0.2% error.  Weights are pre-scaled (x32) on
    the host to keep fp8 out of the subnormal range; the q copy un-scales
    by 1/1024.
  - v is computed token-major in bf16 (stationary = x^T chunk); Wv carries
    a 1/16 scale so ctx lands in good fp8 range for the output projection.
  - scores for a group of 8 windows (128 tokens) are computed as a dense
    [128,128] block per head; 4 heads share one [128,512] PSUM bank.
    Sigmoid runs on the scalar engine (cast to bf16), then a block-diagonal
    0/1 mask multiply on the vector engine zeroes cross-window garbage.
  - ctx^T = (masked probs)^T-contraction against v, accumulated per
    head-pair into one [128,512] PSUM bank (col-packed heads).  The
    PSUM->SBUF step emits an fp8 hi copy plus an fp8 residual lo
    (vector-engine subtract), laid out in DoubleRow stationary form.
  - the output projection runs as 6 fp8 DoubleRow matmuls per group
    (ctx_h*Woh + ctx_h*Wol + ctx_l*Woh, Wo pre-scaled x16 to cancel v's
    1/16); y is written to DRAM in bf16 and cast to f32 on the host.

Biases are folded in only when nonzero (the spec fills them with zeros).
"""

import numpy as np
import ml_dtypes

# ---- problem constants (hardcoded per the task contract) ----
N_CORES = 8
B, NH, NW, T, D = 16, 16, 16, 16, 512
HEADS, HS = 8, 64
TOK = (B // N_CORES) * NH * NW * T  # 8192 tokens per core
NG = TOK // 512                     # 16 supergroups of 512 tokens
SCALE = 1.0 / 8.0                   # 1/sqrt(HS)
WQK_S = 32.0                        # fp8 range scale on Wq/Wk
WV_S = 1.0 / 16.0                   # scale on Wv (ctx -> fp8 range)
WO_S = 16.0                         # scale on Wo (cancels WV_S)
PM = 144                            # padded DoubleRow stationary stride
PN = 528                            # padded DoubleRow moving stride

_CACHE = {}


def _build(n_cores, with_bq, with_bk, with_bv, with_bo):
    import concourse.bacc as bacc
    import concourse.mybir as mybir
    import concourse.tile as tile

    f32 = mybir.dt.float32
    f32r = mybir.dt.float32r
    bf16 = mybir.dt.bfloat16
    fp8 = mybir.dt.float8e4
    AFT = mybir.ActivationFunctionType
    DR = mybir.MatmulPerfMode.DoubleRow
    SUB = mybir.AluOpType.subtract

    nc = bacc.Bacc("TRN2", target_bir_lowering=False, debug=False,
                   num_devices=n_cores)

    xh_d = nc.dram_tensor("xh", [D, TOK], fp8, kind="ExternalInput").ap()
    xl_d = nc.dram_tensor("xl", [D, TOK], fp8, kind="ExternalInput").ap()
    w8_d = {}
    for name in ("wqh", "wql", "wkh", "wkl"):
        w8_d[name] = nc.dram_tensor(name, [128, 4 * 2 * 2 * PM], fp8,
                                    kind="ExternalInput").ap()
    for name in ("woh", "wol", "wvh", "wvl"):
        w8_d[name] = nc.dram_tensor(name, [128, 2 * 2 * PN], fp8,
                                    kind="ExternalInput").ap()
    mask_d = nc.dram_tensor("maskp", [128, 512], bf16,
                            kind="ExternalInput").ap()
    bias_d = {}
    for name, used, dt_b in (("bq", with_bq, f32), ("bk", with_bk, f32),
                             ("bv", with_bv, bf16), ("bo", with_bo, f32r)):
        if used:
            bias_d[name] = nc.dram_tensor(name, [D], dt_b,
                                          kind="ExternalInput").ap()
    y_d = nc.dram_tensor("y", [TOK, D], bf16, kind="ExternalOutput").ap()

    with tile.TileContext(nc) as tc:
        with (
            tc.tile_pool(name="const", bufs=1) as cpool,
            tc.tile_pool(name="work", bufs=2) as wpool,
            tc.tile_pool(name="psum", bufs=4, space="PSUM") as ppool,
        ):
            # ---- constants ----
            w8 = {}
            for name in ("wqh", "wql", "wkh", "wkl", "woh", "wol",
                         "wvh", "wvl"):
                cols = 4 * 2 * 2 * PM if name[1] == "q" or name[1] == "k" \
                    else 2 * 2 * PN
                t_w = cpool.tile([128, cols], fp8, name=f"{name}_sb")
                nc.scalar.dma_start(out=t_w[:], in_=w8_d[name][:])
                w8[name] = t_w
            mask_sb = cpool.tile([128, 512], bf16, name="mask_sb")
            nc.scalar.dma_start(out=mask_sb[:], in_=mask_d[:])
            bias_sb = {}
            for name, ap_d in bias_d.items():
                if name not in ("bq", "bk"):
                    continue
                b_t = cpool.tile([128, 4], f32, name=f"{name}_sb")
                nc.scalar.dma_start(
                    out=b_t[:],
                    in_=ap_d.rearrange("(c p) -> p c", p=128))
                bias_sb[name] = b_t
            ones_sb = ones_bf_sb = None
            if with_bo:
                ones_sb = cpool.tile([1, 128], f32r, name="ones_sb")
                nc.gpsimd.memset(ones_sb[:], 1.0)
            if with_bv:
                ones_bf_sb = cpool.tile([1, 128], bf16, name="ones_bf_sb")
                nc.gpsimd.memset(ones_bf_sb[:], 1.0)
            bvrow_sb = bohrow_sb = None
            if with_bv:
                bvrow_sb = cpool.tile([1, 512], bf16, name="bvrow_sb")
                nc.scalar.dma_start(out=bvrow_sb[:],
                                    in_=bias_d["bv"].unsqueeze(0))
            if with_bo:
                bohrow_sb = cpool.tile([1, 512], f32r, name="bohrow_sb")
                nc.scalar.dma_start(out=bohrow_sb[:],
                                    in_=bias_d["bo"].unsqueeze(0))

            def w_qk_slice(name, c, P):
                v = w8[name].rearrange("p (c P t m) -> p c P t m",
                                       c=4, P=2, m=PM)
                return v[:, c, P, :, 0:128]

            def wo_slice(name, Q):
                v = w8[name].rearrange("p (Q t n) -> p Q t n", Q=2, n=PN)
                return v[:, Q, :, 0:512]

            # ---- per-supergroup emitters (2-stage software pipeline) ----
            def load_x(G):
                """Host pre-transposed x (fp8 hi/lo): one DMA per copy."""
                x8t = []
                for nm, d_ap in (("xph", xh_d), ("xpl", xl_d)):
                    t8 = wpool.tile([128, 4 * PN], fp8, name=nm, tag=nm)
                    nc.sync.dma_start(
                        out=t8.rearrange("p (c t) -> p c t",
                                         t=PN)[:, :, 0:512],
                        in_=d_ap.rearrange("(c p) t -> p c t",
                                           p=128)[:, :, G * 512:(G + 1) * 512])
                    x8t.append(t8)
                return x8t[0], x8t[1]

            def x_pair(t8, P):
                v = t8.rearrange("p (P t n) -> p P t n", P=2, n=PN)
                return v[:, P, :, 0:512]

            def proj_qk_chunk(xh_t, xl_t, wname, bname, dst, c):
                """One 512-token feature chunk of q^T or k^T via 6 fp8
                DoubleRow matmuls (hi*hi + hi*lo + lo*hi)."""
                pj_ps = ppool.tile([128, 512], f32, name="pj_ps", tag="ps")
                for P in range(2):
                    prods = ((f"{wname}h", xh_t), (f"{wname}h", xl_t),
                             (f"{wname}l", xh_t))
                    for i, (wn, xt8) in enumerate(prods):
                        nc.tensor.matmul(
                            pj_ps[:],
                            w_qk_slice(wn, c, P),
                            x_pair(xt8, P),
                            start=(P == 0 and i == 0),
                            stop=(P == 1 and i == 2),
                            perf_mode=DR)
                if bname in bias_sb:
                    sc = 1.0 / 1024.0 if wname == "wq" else 1.0
                    nc.scalar.activation(
                        dst[c][:], pj_ps[:], AFT.Identity,
                        bias=bias_sb[bname][:, c:c + 1], scale=sc)
                elif wname == "wq":
                    if c % 2 == 0:
                        nc.vector.tensor_scalar_mul(dst[c][:], pj_ps[:],
                                                    1.0 / 1024.0)
                    else:
                        nc.scalar.activation(dst[c][:], pj_ps[:],
                                             AFT.Identity,
                                             scale=1.0 / 1024.0)
                elif c % 2 == 0:
                    nc.vector.tensor_copy(dst[c][:], pj_ps[:])
                else:
                    nc.scalar.copy(dst[c][:], pj_ps[:])

            def x_tok_slice(t8, P, g):
                v = t8.rearrange("p (P t n) -> p P t n", P=2, n=PN)
                return v[:, P, :, g * 128:(g + 1) * 128]

            def proj_v(xh_t, xl_t):
                v = [wpool.tile([128, 512], bf16, name=f"v{g}", tag=f"v{g}")
                     for g in range(4)]
                for g in range(4):
                    v_ps = ppool.tile([128, 512], f32, name="v_ps", tag="ps")
                    for P in range(2):
                        prods = ((xh_t, "wvh"), (xh_t, "wvl"), (xl_t, "wvh"))
                        for i, (xt8, wn) in enumerate(prods):
                            nc.tensor.matmul(
                                v_ps[:],
                                x_tok_slice(xt8, P, g),
                                wo_slice(wn, P),
                                start=(P == 0 and i == 0),
                                stop=(P == 1 and i == 2 and not with_bv),
                                perf_mode=DR)
                    if with_bv:
                        nc.tensor.matmul(v_ps[:], ones_bf_sb[:],
                                         bvrow_sb[:],
                                         start=False, stop=True)
                    if g % 2 == 0:
                        nc.vector.tensor_scalar_mul(v[g][:], v_ps[:],
                                                    1.0 / 512.0)
                    else:
                        nc.scalar.activation(v[g][:], v_ps[:], AFT.Identity,
                                             scale=1.0 / 512.0)
                return v

            def scores(P, qt, kt, g):
                """S' matmuls + sigmoid + mask for one 128-token group."""
                p4 = []
                for half in range(2):  # even heads / odd heads
                    s_ps = ppool.tile([128, 512], f32, name="s_ps", tag="s",
                                      bufs=4)
                    lo = half * 64
                    for hh in range(4):
                        h = 2 * hh + half
                        c = h // 2
                        gcols = slice(g * 128, (g + 1) * 128)
                        nc.tensor.matmul(
                            s_ps[:, hh * 128:(hh + 1) * 128],
                            kt[c][lo:lo + 64, gcols],
                            qt[c][lo:lo + 64, gcols],
                            start=True, stop=True)
                    p_t = wpool.tile([128, 512], bf16, name=f"p{g}_{half}",
                                     tag=f"p{g}_{half}")
                    nc.scalar.activation(p_t[:], s_ps[:], AFT.Sigmoid)
                    nc.vector.tensor_mul(
                        p_t.rearrange("p (hh t) -> p hh t", hh=4),
                        p_t.rearrange("p (hh t) -> p hh t", hh=4),
                        mask_sb.rearrange("p (hh t) -> p hh t", hh=4))
                    p4.append(p_t)
                return p4

            def ctx_out(P, pr, v):
                ctx8 = []
                for g in range(4):
                    ctx_ps = ppool.tile([128, 512], f32, name="ctx_ps",
                                        tag="ps")
                    for h in range(HEADS):
                        c, lo = h // 2, (h % 2) * 64
                        nc.tensor.matmul(
                            ctx_ps[lo:lo + 64, c * 128:(c + 1) * 128],
                            v[g][:, h * 64:(h + 1) * 64],
                            pr[g][h % 2][:, (h // 2) * 128:
                                          (h // 2 + 1) * 128],
                            start=True, stop=True)
                    # fp8 hi + residual lo in DoubleRow stationary layout
                    c_h = wpool.tile([128, 2 * 2 * PM], fp8, name="cth",
                                     tag=f"cth{g}", bufs=2)
                    c_l = wpool.tile([128, 2 * 2 * PM], fp8, name="ctl",
                                     tag=f"ctl{g}", bufs=2)
                    hv = c_h.rearrange("p (Q t m) -> p Q t m",
                                       Q=2, m=PM)[:, :, :, 0:128]
                    lv = c_l.rearrange("p (Q t m) -> p Q t m",
                                       Q=2, m=PM)[:, :, :, 0:128]
                    src = ctx_ps.rearrange("p (Q t q) -> p Q t q", Q=2, t=2)
                    if g % 2 == 0:
                        nc.scalar.copy(hv, src)
                    else:
                        nc.vector.tensor_copy(hv, src)
                    nc.vector.tensor_tensor(lv, src, hv, SUB)
                    ctx8.append((c_h, c_l))
                for g in range(4):
                    o_ps = ppool.tile([128, 512], f32, name="o_ps", tag="ps")
                    c_h, c_l = ctx8[g]
                    for Q in range(2):
                        prods = ((c_h, "woh"), (c_h, "wol"), (c_l, "woh"))
                        for i, (ct, wn) in enumerate(prods):
                            st = ct.rearrange("p (Q t m) -> p Q t m",
                                              Q=2, m=PM)[:, Q, :, 0:128]
                            nc.tensor.matmul(
                                o_ps[:], st, wo_slice(wn, Q),
                                start=(Q == 0 and i == 0),
                                stop=(Q == 1 and i == 2 and not with_bo),
                                perf_mode=DR)
                    if with_bo:
                        nc.tensor.matmul(o_ps[:], ones_sb[:], bohrow_sb[:],
                                         start=False, stop=True)
                    o_t = wpool.tile([128, 512], bf16, name="o_t", tag="o_t",
                                     bufs=4)
                    if g % 2 == 0:
                        nc.scalar.copy(o_t[:], o_ps[:])
                    else:
                        nc.vector.tensor_copy(o_t[:], o_ps[:])
                    nc.scalar.dma_start(
                        out=y_d[(P * 4 + g) * 128:(P * 4 + g + 1) * 128, :],
                        in_=o_t[:])

            # ---- pipelined emission: stage A(G) interleaved with B(G-1) ----
            x_next = load_x(0)
            prev = None  # (P, qt, kt, v)
            for G in range(NG + 1):
                if G < NG:
                    xh_t, xl_t = x_next
                if G + 1 < NG:
                    x_next = load_x(G + 1)
                pr = []
                if G < NG:
                    qt = [wpool.tile([128, 512], bf16, name=f"wqt{c}",
                                     tag=f"wqt{c}") for c in range(4)]
                    kt = [wpool.tile([128, 512], bf16, name=f"wkt{c}",
                                     tag=f"wkt{c}") for c in range(4)]
                for g in range(4):
                    if prev is not None:
                        pr.append(scores(prev[0], prev[1], prev[2], g))
                    if G < NG:
                        proj_qk_chunk(xh_t, xl_t, "wq", "bq", qt, g)
                        proj_qk_chunk(xh_t, xl_t, "wk", "bk", kt, g)
                if G < NG:
                    v = proj_v(xh_t, xl_t)
                if prev is not None and pr:
                    ctx_out(prev[0], pr, prev[3])
                prev = (G, qt, kt, v) if G < NG else None

    nc.compile()
    return nc


def _get_nc(n_cores, flags):
    key = (n_cores, flags)
    if key not in _CACHE:
        _CACHE[key] = _build(n_cores, *flags)
    return _CACHE[key]


def _mask4():
    m = np.zeros((128, 128), dtype=ml_dtypes.bfloat16)
    for w in range(8):
        m[w * 16:(w + 1) * 16, w * 16:(w + 1) * 16] = 1
    return np.ascontiguousarray(np.tile(m, (1, 4)))


def _fp8_split(a, np8):
    hi = a.astype(np8)
    lo = (a - hi.astype(np.float32)).astype(np8)
    return hi, lo


def _pack_wqk(w, np8):
    """[512, 512] -> [128, (c4, P2, t2, PM)] DoubleRow stationary layout."""
    out = np.zeros((128, 4, 2, 2, PM), np.float32)
    for c in range(4):
        for P in range(2):
            for t in range(2):
                blk = w[(2 * P + t) * 128:(2 * P + t + 1) * 128,
                        c * 128:(c + 1) * 128]
                out[:, c, P, t, 0:128] = blk
    return _fp8_split(np.ascontiguousarray(out.reshape(128, -1)), np8)


def _pack_wo(w):
    """[512, 512] -> [128, (Q2, t2, PN)] DoubleRow moving layout."""
    out = np.zeros((128, 2, 2, PN), np.float32)
    for Q in range(2):
        for t in range(2):
            out[:, Q, t, 0:512] = w[(2 * Q + t) * 128:(2 * Q + t + 1) * 128, :]
    return np.ascontiguousarray(out.reshape(128, -1))


def host_prep(x, Wq, bq, Wk, bk, Wv, bv, Wo, bo, flags):
    """Build the per-core device input dicts."""
    import concourse.mybir as mybir

    np8 = mybir.dt.np(mybir.dt.float8e4)
    # feature-major (pre-transposed) x copies
    xT = np.ascontiguousarray(
        np.asarray(x, np.float32).reshape(N_CORES, TOK, D)
        .transpose(0, 2, 1))                       # [cores, D, TOK]
    xh = xT.astype(np8)
    xl = (xT - xh.astype(np.float32)).astype(np8)

    wqh, wql = _pack_wqk(np.asarray(Wq, np.float32) * (SCALE * WQK_S), np8)
    wkh, wkl = _pack_wqk(np.asarray(Wk, np.float32) * WQK_S, np8)
    woh, wol = _fp8_split(_pack_wo(np.asarray(Wo, np.float32) * WO_S), np8)
    wvh, wvl = _fp8_split(_pack_wo(np.asarray(Wv, np.float32) * WQK_S), np8)
    base = {
        "wqh": wqh, "wql": wql, "wkh": wkh, "wkl": wkl,
        "woh": woh, "wol": wol, "wvh": wvh, "wvl": wvl,
        "maskp": _mask4(),
    }
    for name, b, used, scale in (("bq", bq, flags[0], SCALE),
                                 ("bk", bk, flags[1], WQK_S),
                                 ("bv", bv, flags[2], 512.0 * WV_S),
                                 ("bo", bo, flags[3], 1.0)):
        if used:
            arr = np.asarray(b, np.float32) * scale
            if name == "bv":
                arr = arr.astype(ml_dtypes.bfloat16)
            base[name] = np.ascontiguousarray(arr)

    return [dict(base, xh=np.ascontiguousarray(xh[i]),
                 xl=np.ascontiguousarray(xl[i]))
            for i in range(N_CORES)]


def kernel(x, Wq, bq, Wk, bk, Wv, bv, Wo, bo):
    from concourse.bass_utils import run_bass_kernel_spmd

    in_dt = x.dtype
    flags = tuple(bool(np.any(b)) for b in (bq, bk, bv, bo))
    nc = _get_nc(N_CORES, flags)
    in_maps = host_prep(x, Wq, bq, Wk, bk, Wv, bv, Wo, bo, flags)
    res = run_bass_kernel_spmd(nc, in_maps, list(range(N_CORES)))
    out = np.concatenate([res.results[i]["y"] for i in range(N_CORES)], axis=0)
    return (out.reshape(B, NH, NW, T, D).astype(np.float32)
            .astype(in_dt, copy=False))



# revision 51
# speedup vs baseline: 1.0009x; 1.0009x over previous
"""Trainium2 Bass kernel for windowed sigmoid-attention (nn_Attention_24927990186215).

Reference computation (per full input):
    x: [16, 16, 16, 16, 512]  (b, nh, nw, t, d) -- windows of T=16 tokens
    q/k/v = x @ W{q,k,v} + b{q,k,v}; split into H=8 heads of 64
    scores = q @ k^T / sqrt(64) within each 16-token window
    probs = sigmoid(scores)  (elementwise, NOT softmax)
    ctx = probs @ v;  out = ctx @ Wo + bo
Sharding: data-parallel over batch dim (16) across 8 cores -> 2 batches
(8192 tokens) per core.

Per-core dataflow (all matmuls on the PE):
  - host ships a merged feature-major fp8 hi/lo residual copy of x
    (x ~ xh + xl) plus one concatenated DoubleRow-packed fp8 hi/lo
    weight blob.  q^T/k^T/v are computed with fp8 DoubleRow matmuls
    (three residual cross terms; the xl*Wl term is dropped, ~0.1% err).
  - scores exploit the block-diagonal (windowed) structure: each window's
    [8 heads x 16 x 16] score block is computed PACKED, partition=(h,s),
    cols=t.  Heads 0-3 land in one [64,512] psum tile, heads 4-7 in a
    second, so each chunk's band starts at partition 0 or 32 (the hw only
    allows matmul output bases 0/32/64).  4 independent bf16 matmuls of
    16 output columns per window against half-zero "kslab" stationaries
    built from k^T.  2x fewer PE columns than dense [128,128] blocks,
    sigmoid runs once per [64,512] tile (8x less activation work), no
    mask multiply at all.
  - v bounces through DRAM (SBUF partition-split DMA sources are
    unreliable) and is gathered into "vpad" stationary tiles
    [32 (h,s) rows, (q,d) cols] -- pure data, no zero padding -- so
    ctx^T is computed packed too: one [32-contraction, 16-col] matmul
    per (head-pair, window) accumulating feature-major ctx^T tiles
    [128 (h,d), 512 tokens].  The DRAM bounce write and its dependent
    gathers share one engine queue (FIFO) because DRAM RAW deps are not
    tracked by the tile framework.
  - ctx^T converts to fp8 hi+lo directly in DoubleRow stationary form and
    the output projection runs as 6 fp8 DR matmuls per 128-token group;
    y is staged in bf16 and written with two DMAs per 512 tokens.
  - dependency-tracker constraint honored throughout: all repeated
    writers of one SBUF tile use a single engine (readers only inherit
    the last writing engine per region).

Biases are folded in only when nonzero (the spec fills them with zeros).
"""

import numpy as np
import ml_dtypes

# ---- problem constants (hardcoded per the task contract) ----
N_CORES = 8
B, NH, NW, T, D = 16, 16, 16, 16, 512
HEADS, HS = 8, 64
TOK = (B // N_CORES) * NH * NW * T  # 8192 tokens per core
NG = TOK // 512                     # 16 supergroups of 512 tokens
SCALE = 1.0 / 8.0                   # 1/sqrt(HS)
WQK_S = 32.0                        # fp8 range scale on Wq/Wk
WO_S = 16.0                         # scale on Wo (cancels v's 1/16)
PM = 144                            # padded DoubleRow stationary stride
PN = 528                            # padded DoubleRow moving stride
WQK_COLS = 4 * 2 * 2 * PM           # 2304
WVO_COLS = 2 * 2 * PN               # 2112
WALL_COLS = 2 * WQK_COLS + 2 * WVO_COLS  # 8832

_CACHE = {}


def _build(n_cores, with_bq, with_bk, with_bv, with_bo, ng=NG):
    import concourse.bacc as bacc
    import concourse.mybir as mybir
    import concourse.tile as tile

    f32 = mybir.dt.float32
    f32r = mybir.dt.float32r
    bf16 = mybir.dt.bfloat16
    fp16 = mybir.dt.float16
    fp8 = mybir.dt.float8e4
    AFT = mybir.ActivationFunctionType
    DR = mybir.MatmulPerfMode.DoubleRow
    SUB = mybir.AluOpType.subtract

    tok = ng * 512

    nc = bacc.Bacc("TRN2", target_bir_lowering=False, debug=False,
                   num_devices=n_cores)

    def copy_op(eng, dst, src):
        if eng is nc.scalar:
            eng.copy(dst, src)
        else:
            eng.tensor_copy(dst, src)

    # x8: [p, z(hi/lo), c(4), t] merged fp8 residual pair, feature-major
    x8_d = nc.dram_tensor("x8", [128, 2 * 4 * tok], fp8,
                          kind="ExternalInput").ap()
    wall_d = nc.dram_tensor("wall", [128, WALL_COLS], fp8,
                            kind="ExternalInput").ap()
    wob_d = nc.dram_tensor("wob", [128, 4 * 512], fp16,
                           kind="ExternalInput").ap()
    bias_d = {}
    for name, used, dt_b in (("bq", with_bq, f32), ("bk", with_bk, f32),
                             ("bv", with_bv, bf16), ("bo", with_bo, f32r)):
        if used:
            bias_d[name] = nc.dram_tensor(name, [D], dt_b,
                                          kind="ExternalInput").ap()
    # DRAM bounce for the v -> vpad partition shuffle (double phase)
    vdr_d = nc.dram_tensor("vdr", [2 * 512, 512], fp16, kind="Internal").ap()
    y_d = nc.dram_tensor("y", [tok, D], fp16, kind="ExternalOutput").ap()

    with tile.TileContext(nc) as tc:
        with (
            tc.tile_pool(name="const", bufs=1) as cpool,
            tc.tile_pool(name="work", bufs=2) as wpool,
            tc.tile_pool(name="psum", bufs=5, space="PSUM") as ppool,
            tc.tile_pool(name="attn", bufs=3, space="PSUM") as apool,
        ):
            # ---- constants: three DMAs for all weights (q/k part first
            # so the opening projections start sooner) ----
            wall_sb = cpool.tile([128, WALL_COLS], fp8, name="wall_sb")
            nqk = 2 * WQK_COLS
            nc.scalar.dma_start(out=wall_sb[:, 0:WQK_COLS],
                                in_=wall_d[:, 0:WQK_COLS])
            nc.scalar.dma_start(out=wall_sb[:, WQK_COLS:nqk],
                                in_=wall_d[:, WQK_COLS:nqk])
            nc.scalar.dma_start(out=wall_sb[:, nqk:WALL_COLS],
                                in_=wall_d[:, nqk:WALL_COLS])
            wob_sb = cpool.tile([128, 4 * 512], fp16, name="wob_sb")
            nc.scalar.dma_start(out=wob_sb[:], in_=wob_d[:])
            woff = {}
            off = 0
            for name in ("wqh", "wkh"):
                woff[name] = off
                off += WQK_COLS
            for name in ("wvh", "wvl"):
                woff[name] = off
                off += WVO_COLS

            bias_sb = {}
            for name, ap_d in bias_d.items():
                if name not in ("bq", "bk"):
                    continue
                b_t = cpool.tile([128, 4], f32, name=f"{name}_sb")
                nc.scalar.dma_start(
                    out=b_t[:],
                    in_=ap_d.rearrange("(c p) -> p c", p=128))
                bias_sb[name] = b_t
            ones_sb = ones_bf_sb = None
            if with_bo:
                ones_sb = cpool.tile([1, 128], f32r, name="ones_sb")
                nc.gpsimd.memset(ones_sb[:], 1.0)
            if with_bv:
                ones_bf_sb = cpool.tile([1, 128], bf16, name="ones_bf_sb")
                nc.gpsimd.memset(ones_bf_sb[:], 1.0)
            bvrow_sb = bohrow_sb = None
            if with_bv:
                bvrow_sb = cpool.tile([1, 512], bf16, name="bvrow_sb")
                nc.scalar.dma_start(out=bvrow_sb[:],
                                    in_=bias_d["bv"].unsqueeze(0))
            if with_bo:
                bohrow_sb = cpool.tile([1, 512], f32r, name="bohrow_sb")
                nc.scalar.dma_start(out=bohrow_sb[:],
                                    in_=bias_d["bo"].unsqueeze(0))

            # kslab[c]: [128 feat, (z, 32w, 2j, 16s)] bf16 -- stationary for
            # packed scores; col (j,s) is nonzero on rows j*64..j*64+63 only,
            # the other half is zeroed once here and never overwritten.
            kslab = [cpool.tile([128, 2 * 1024], fp16, name=f"ks{c}")
                     for c in range(4)]
            for c in range(4):
                v = kslab[c].rearrange("p (z w j s) -> p z w j s",
                                       z=2, w=32, j=2, s=16)
                nc.vector.memset(v[0:64, :, :, 1, :], 0.0)
                nc.vector.memset(v[64:128, :, :, 0, :], 0.0)
            # vpad_lo/hi: [64, (z, 8wl, 2q, 4g, 64d)] bf16 -- ctx stationary
            # bands at rows (2m+q)%4*16+s.  Each 32-row band is 2x2 block-
            # diagonal in (row-half, q): the off-diagonal quadrants must be
            # zero; they are zeroed once here and never overwritten.
            vpad = [cpool.tile([64, 2 * 4096], fp16, name=f"vp{j}")
                    for j in range(2)]
            # engine ops cannot start at partition 16, so zero-fill the
            # quadrants by DMA from a zeroed staging tile; same sync queue
            # as the band-gather writers keeps ordering trivially correct.
            zpad = cpool.tile([16, 4096], fp16, name="zpad")
            nc.vector.memset(zpad[:], 0.0)

            def emit_vpad_zero_fills(z):
                zsrc = zpad.rearrange("p (a b) -> p a b", b=64)
                for j in range(2):
                    for r in range(4):
                        dstz = vpad[j].rearrange(
                            "(r s) (z gwl q d) -> r s z gwl q d",
                            s=16, z=2, gwl=32, q=2, d=64)[
                                r, :, z, :, 1 - (r % 2)]
                        nc.sync.dma_start(out=dstz, in_=zsrc[:, 0:32])

            def w_qk_slice(name, c, P):
                v = wall_sb[:, woff[name]:woff[name] + WQK_COLS].rearrange(
                    "p (c P t m) -> p c P t m", c=4, P=2, m=PM)
                return v[:, c, P, :, 0:128]

            def wo_slice(name, Q):
                v = wall_sb[:, woff[name]:woff[name] + WVO_COLS].rearrange(
                    "p (Q t n) -> p Q t n", Q=2, n=PN)
                return v[:, Q, :, 0:512]

            # ---- per-supergroup emitters (2-stage software pipeline) ----
            def load_x(G):
                """One DMA for the merged hi/lo feature-major x slice."""
                t8 = wpool.tile([128, 8 * PN], fp8, name="x8t", tag="x8t")
                nc.sync.dma_start(
                    out=t8.rearrange("p (zc n) -> p zc n", n=PN)[:, :, 0:512],
                    in_=x8_d.rearrange("p (zc t) -> p zc t", zc=8)[
                        :, :, G * 512:(G + 1) * 512])
                return t8

            def x_pair(t8, z, P):
                v = t8.rearrange("p (z P t n) -> p z P t n", z=2, P=2, n=PN)
                return v[:, z, P, :, 0:512]

            def proj_qk_mms(t8, wname, pj_ps):
                # 2-term residual: full-x (hi+lo) against W-hi only.  The
                # dropped x*Wl term costs ~1e-2 relative error end-to-end
                # (sigmoid damps the scores path), well under the 2e-2 gate.
                for P in range(2):
                    for z in range(2):
                        nc.tensor.matmul(
                            pj_ps[:], w_qk_slice(f"{wname}h", c_cur[0], P),
                            x_pair(t8, z, P),
                            start=(P == 0 and z == 0),
                            stop=(P == 1 and z == 1),
                            perf_mode=DR)

            c_cur = [0]

            def proj_q_chunk(t8, dst, c):
                c_cur[0] = c
                pj_ps = ppool.tile([128, 512], f32, name="pj_ps", tag="ps")
                proj_qk_mms(t8, "wq", pj_ps)
                if "bq" in bias_sb:
                    nc.scalar.activation(
                        dst[c][:], pj_ps[:], AFT.Identity,
                        bias=bias_sb["bq"][:, c:c + 1], scale=1.0 / 1024.0)
                else:
                    nc.vector.tensor_scalar_mul(dst[c][:], pj_ps[:],
                                                1.0 / 1024.0)

            def proj_k_chunk(t8, c, ph):
                """k^T chunk -> half-zero kslab stationary blocks."""
                c_cur[0] = c
                pj_ps = ppool.tile([128, 512], f32, name="pj_ps", tag="ps")
                proj_qk_mms(t8, "wk", pj_ps)
                dst = kslab[c].rearrange("p (z w j s) -> p z w j s",
                                         z=2, w=32, j=2, s=16)[:, ph]
                src = pj_ps.rearrange("p (w s) -> p w s", s=16)
                if "bk" in bias_sb:
                    nc.scalar.activation(
                        dst[0:64, :, 0, :], src[0:64], AFT.Identity,
                        bias=bias_sb["bk"][0:64, c:c + 1])
                    nc.scalar.activation(
                        dst[64:128, :, 1, :], src[64:128],
                        AFT.Identity, bias=bias_sb["bk"][64:128, c:c + 1])
                else:
                    # one engine per kslab tile (dep-tracker constraint)
                    eng = (nc.vector, nc.scalar, nc.vector, nc.scalar)[c]
                    copy_op(eng, dst[0:64, :, 0, :], src[0:64])
                    copy_op(eng, dst[64:128, :, 1, :], src[64:128])

            def proj_v(t8, ph):
                """v (token-major, bf16, 1/16-scaled) -> DRAM bounce ->
                vpad band gathers.  All on the sync queue (FIFO ordering
                substitutes for the untracked DRAM RAW dependency)."""
                v_sb = wpool.tile([128, 2048], fp16, name="v_sb", tag="v_sb")
                xsv = t8.rearrange("p (z P t n) -> p z P t n",
                                   z=2, P=2, n=PN)
                for g in range(4):
                    v_ps = ppool.tile([128, 512], f32, name="v_ps", tag="ps")
                    for P in range(2):
                        st_h = xsv[:, 0, P, :, g * 128:(g + 1) * 128]
                        st_l = xsv[:, 1, P, :, g * 128:(g + 1) * 128]
                        prods = ((st_h, "wvh"), (st_h, "wvl"), (st_l, "wvh"))
                        for i, (st, wn) in enumerate(prods):
                            nc.tensor.matmul(
                                v_ps[:], st, wo_slice(wn, P),
                                start=(P == 0 and i == 0),
                                stop=(P == 1 and i == 2 and not with_bv),
                                perf_mode=DR)
                    if with_bv:
                        nc.tensor.matmul(v_ps[:], ones_bf_sb[:], bvrow_sb[:],
                                         start=False, stop=True)
                    # v_sb cols are (g, hp, q, d): plain contiguous copy
                    nc.scalar.activation(v_sb[:, g * 512:(g + 1) * 512],
                                         v_ps[:], AFT.Identity,
                                         scale=1.0 / 512.0)
                # bounce: vdr row = (ph, g, wl*16+s), col = (hp q d)
                vdrv = vdr_d.rearrange("(z g p) c -> z p g c", z=2, g=4)
                nc.sync.dma_start(
                    out=vdrv[ph],
                    in_=v_sb.rearrange("p (g c) -> p g c", g=4))
                # band gather: dims [s, (g wl), d] on both sides
                srcb = vdr_d.rearrange(
                    "(z gwl s) (hp q d) -> z hp q s gwl d",
                    s=16, z=2, gwl=32, hp=4, q=2, d=64)
                for m in range(4):
                    dstp = vpad[m // 2].rearrange(
                        "(r s) (z gwl q d) -> r z q s gwl d",
                        s=16, z=2, gwl=32, q=2, d=64)
                    for q in range(2):
                        nc.sync.dma_start(
                            out=dstp[2 * (m % 2) + q, ph, q],
                            in_=srcb[ph, m, q])

            def ks_win(c, ph, w):
                """[128, 32] stationary slice of kslab[c], phase ph, win w."""
                return kslab[c].rearrange("p (z w n) -> p z w n",
                                          z=2, n=32)[:, ph, w]

            def scores(ph, qt, sc2):
                """packed scores: 4 independent 16-col matmuls per window."""
                for w in range(32):
                    wl = w % 16
                    cols = slice(wl * 16, (wl + 1) * 16)
                    for c in range(4):
                        sc = sc2[(c // 2) * 2 + w // 16]
                        rows = slice((c % 2) * 32, (c % 2) * 32 + 32)
                        nc.tensor.matmul(
                            sc[rows, cols], ks_win(c, ph, w),
                            qt[c][:, w * 16:(w + 1) * 16],
                            start=True, stop=True)

            def ctx(ph, pb2):
                """packed ctx^T (32-contraction) + fp16 stationary copy."""
                cs = [wpool.tile([128, 2 * 4 * 128], fp16, name=f"cs{Q}",
                                 tag=f"cs{Q}") for Q in range(2)]
                cs4 = [t.rearrange("p (t g n) -> p t g n", t=2, g=4, n=128)
                       for t in cs]
                for m in (2, 3, 0, 1):
                    cx = apool.tile([128, 512], f32, name=f"cx{m}",
                                     tag="at")
                    vpv = vpad[m // 2].rearrange(
                        "r (z gwl qd) -> r z gwl qd", z=2, qd=128)
                    rlo = (m % 2) * 32
                    pb = pb2[m // 2]
                    for w in range(32):
                        nc.tensor.matmul(
                            cx[:, w * 16:(w + 1) * 16],
                            vpv[rlo:rlo + 32, ph, w],
                            pb[rlo:rlo + 32, w * 16:(w + 1) * 16],
                            start=True, stop=True)
                    Q, t = m // 2, m % 2
                    src = cx.rearrange("p (g n) -> p g n", g=4)
                    # one engine per cs tile (dep-tracker constraint);
                    # GPSIMD cannot read PSUM, so only ACT/DVE qualify.
                    eng = nc.scalar if Q == 0 else nc.vector
                    copy_op(eng, cs4[Q][:, t], src)
                return cs4

            def out_proj(cs4, y_base):
                ot = [wpool.tile([128, 1024], fp16, name=f"ot{j}",
                                 tag=f"ot{j}") for j in range(2)]
                for g in range(4):
                    o_ps = ppool.tile([128, 512], f32, name="o_ps", tag="ps")
                    for m in range(4):
                        nc.tensor.matmul(
                            o_ps[:], cs4[m // 2][:, m % 2, g, :],
                            wob_sb[:, m * 512:(m + 1) * 512],
                            start=(m == 0),
                            stop=(m == 3 and not with_bo))
                    if with_bo:
                        nc.tensor.matmul(o_ps[:], ones_sb[:], bohrow_sb[:],
                                         start=False, stop=True)
                    dst = ot[g // 2][:, (g % 2) * 512:(g % 2) * 512 + 512]
                    eng = nc.vector
                    copy_op(eng, dst, o_ps[:])
                for j in range(2):
                    nc.scalar.dma_start(
                        out=y_d.rearrange("(r p) n -> p r n", p=128)[
                            :, 2 * j + y_base // 128:
                            2 * j + 2 + y_base // 128, :],
                        in_=ot[j].rearrange("p (r n) -> p r n", r=2))

            # ---- pipelined emission ----
            # per iteration: scores(P) -> sigmoid(P) -> proj_qk(G) ->
            # ctx(P) -> proj_v(G)+vpad DMAs(G) -> out_proj(P).
            x_next = load_x(0)
            prev = None  # (ph, qt)
            for G in range(ng + 1):
                ph = G % 2
                if G < ng:
                    t8 = x_next
                if G < 2:
                    emit_vpad_zero_fills(G)
                if prev is not None:
                    # four half-width score tiles -> 8 allocations per
                    # iteration on the 4-buffer "at" tag, so scores always
                    # reuse buffers freed in the PREVIOUS iteration.
                    sc2 = [apool.tile([64, 256], f32, name=f"sc{i}",
                                       tag="at") for i in range(4)]
                    pb2 = [wpool.tile([64, 512], fp16, name=f"pb{i}",
                                      tag=f"pb{i}") for i in range(2)]
                    scores(prev[0], prev[1], sc2)
                    for i in range(4):
                        nc.scalar.activation(
                            pb2[i // 2][:, (i % 2) * 256:(i % 2) * 256 + 256],
                            sc2[i][:], AFT.Sigmoid)
                if prev is not None:
                    cs4 = ctx(prev[0], pb2)
                if G < ng:
                    qt = [wpool.tile([128, 512], fp16, name=f"wqt{c}",
                                     tag=f"wqt{c}") for c in range(4)]
                    for c in range(4):
                        proj_q_chunk(t8, qt, c)
                        proj_k_chunk(t8, c, ph)
                    proj_v(t8, ph)
                    if G + 1 < ng:
                        x_next = load_x(G + 1)
                if prev is not None:
                    out_proj(cs4, (G - 1) * 512)
                prev = (ph, qt) if G < ng else None

    nc.compile()
    return nc


def _get_nc(n_cores, flags):
    key = (n_cores, flags)
    if key not in _CACHE:
        _CACHE[key] = _build(n_cores, *flags)
    return _CACHE[key]


def _fp8_split(a, np8):
    hi = a.astype(np8)
    lo = (a - hi.astype(np.float32)).astype(np8)
    return hi, lo


def _pack_wqk(w, np8):
    """[512, 512] -> [128, (c4, P2, t2, PM)] DoubleRow stationary layout."""
    out = np.zeros((128, 4, 2, 2, PM), np.float32)
    for c in range(4):
        for P in range(2):
            for t in range(2):
                blk = w[(2 * P + t) * 128:(2 * P + t + 1) * 128,
                        c * 128:(c + 1) * 128]
                out[:, c, P, t, 0:128] = blk
    return _fp8_split(np.ascontiguousarray(out.reshape(128, -1)), np8)


def _pack_wo(w):
    """[512, 512] -> [128, (Q2, t2, PN)] DoubleRow moving layout."""
    out = np.zeros((128, 2, 2, PN), np.float32)
    for Q in range(2):
        for t in range(2):
            out[:, Q, t, 0:512] = w[(2 * Q + t) * 128:(2 * Q + t + 1) * 128, :]
    return np.ascontiguousarray(out.reshape(128, -1))


def host_prep(x, Wq, bq, Wk, bk, Wv, bv, Wo, bo, flags):
    """Build the per-core device input dicts."""
    import concourse.mybir as mybir

    np8 = mybir.dt.np(mybir.dt.float8e4)
    # feature-major (pre-transposed) x copies, merged [p, z, c, t]
    xT = np.ascontiguousarray(
        np.asarray(x, np.float32).reshape(N_CORES, TOK, D)
        .transpose(0, 2, 1))                       # [cores, D, TOK]
    xh = xT.astype(np8)
    xl = (xT - xh.astype(np.float32)).astype(np8)
    # [cores, (c p), t] -> [cores, p, z, c, t]
    x8 = np.stack([xh.reshape(N_CORES, 4, 128, TOK),
                   xl.reshape(N_CORES, 4, 128, TOK)], axis=1)
    x8 = np.ascontiguousarray(x8.transpose(0, 3, 1, 2, 4)  # c,z? ->p,z,c,t
                              .reshape(N_CORES, 128, 2 * 4 * TOK))

    wqh, _ = _pack_wqk(np.asarray(Wq, np.float32) * (SCALE * WQK_S), np8)
    wkh, _ = _pack_wqk(np.asarray(Wk, np.float32) * WQK_S, np8)
    wvh, wvl = _fp8_split(_pack_wo(np.asarray(Wv, np.float32) * WQK_S), np8)
    wall = np.concatenate([wqh, wkh, wvh, wvl], axis=1)
    wall = np.ascontiguousarray(wall)
    # fp16 Wo moving operand: [128, (m, 512)], rows grouped by 128-tile
    wob = np.ascontiguousarray(
        (np.asarray(Wo, np.float32) * WO_S).reshape(4, 128, 512)
        .transpose(1, 0, 2).reshape(128, 4 * 512).astype(np.float16))
    base = {"wall": wall, "wob": wob}
    for name, b, used, scale in (("bq", bq, flags[0], SCALE),
                                 ("bk", bk, flags[1], WQK_S),
                                 ("bv", bv, flags[2], 512.0 / 16.0),
                                 ("bo", bo, flags[3], 1.0)):
        if used:
            arr = np.asarray(b, np.float32) * scale
            if name == "bv":
                arr = arr.astype(ml_dtypes.bfloat16)
            base[name] = np.ascontiguousarray(arr)

    return [dict(base, x8=x8[i]) for i in range(N_CORES)]


def kernel(x, Wq, bq, Wk, bk, Wv, bv, Wo, bo):
    from concourse.bass_utils import run_bass_kernel_spmd

    in_dt = x.dtype
    flags = tuple(bool(np.any(b)) for b in (bq, bk, bv, bo))
    nc = _get_nc(N_CORES, flags)
    in_maps = host_prep(x, Wq, bq, Wk, bk, Wv, bv, Wo, bo, flags)
    res = run_bass_kernel_spmd(nc, in_maps, list(range(N_CORES)))
    out = np.concatenate([res.results[i]["y"] for i in range(N_CORES)], axis=0)
    return (out.reshape(B, NH, NW, T, D).astype(np.float32)
            .astype(in_dt, copy=False))


# revision 62
# speedup vs baseline: 1.0178x; 1.0169x over previous
"""Trainium2 Bass kernel for windowed sigmoid-attention (nn_Attention_24927990186215).

Reference computation (per full input):
    x: [16, 16, 16, 16, 512]  (b, nh, nw, t, d) -- windows of T=16 tokens
    q/k/v = x @ W{q,k,v} + b{q,k,v}; split into H=8 heads of 64
    scores = q @ k^T / sqrt(64) within each 16-token window
    probs = sigmoid(scores)  (elementwise, NOT softmax)
    ctx = probs @ v;  out = ctx @ Wo + bo
Sharding: data-parallel over batch dim (16) across 8 cores -> 2 batches
(8192 tokens) per core.

Per-core dataflow (all matmuls on the PE):
  - host ships a merged feature-major fp8 hi/lo residual copy of x
    (x ~ xh + xl) plus one concatenated DoubleRow-packed fp8 hi/lo
    weight blob.  q^T/k^T/v are computed with fp8 DoubleRow matmuls
    (three residual cross terms; the xl*Wl term is dropped, ~0.1% err).
  - scores exploit the block-diagonal (windowed) structure: each window's
    [8 heads x 16 x 16] score block is computed PACKED, partition=(h,s),
    cols=t.  Heads 0-3 land in one [64,512] psum tile, heads 4-7 in a
    second, so each chunk's band starts at partition 0 or 32 (the hw only
    allows matmul output bases 0/32/64).  4 independent bf16 matmuls of
    16 output columns per window against half-zero "kslab" stationaries
    built from k^T.  2x fewer PE columns than dense [128,128] blocks,
    sigmoid runs once per [64,512] tile (8x less activation work), no
    mask multiply at all.
  - v bounces through DRAM (SBUF partition-split DMA sources are
    unreliable) and is gathered into "vpad" stationary tiles
    [32 (h,s) rows, (q,d) cols] -- pure data, no zero padding -- so
    ctx^T is computed packed too: one [32-contraction, 16-col] matmul
    per (head-pair, window) accumulating feature-major ctx^T tiles
    [128 (h,d), 512 tokens].  The DRAM bounce write and its dependent
    gathers share one engine queue (FIFO) because DRAM RAW deps are not
    tracked by the tile framework.
  - ctx^T converts to fp16 stationary tiles and the output projection
    runs as 4 chained bf16-rate matmuls per 128-token group against an
    fp16 Wo; y is staged in fp16 and written with two DMAs per 512
    tokens.  Intermediates use fp16 (not bf16) for error headroom: the
    2-term q/k projections land ~1.4e-2 relative error end-to-end.
  - dependency-tracker constraint honored throughout: all repeated
    writers of one SBUF tile use a single engine (readers only inherit
    the last writing engine per region).

Biases are folded in only when nonzero (the spec fills them with zeros).
"""

import numpy as np
import ml_dtypes

# ---- problem constants (hardcoded per the task contract) ----
N_CORES = 8
B, NH, NW, T, D = 16, 16, 16, 16, 512
HEADS, HS = 8, 64
TOK = (B // N_CORES) * NH * NW * T  # 8192 tokens per core
NG = TOK // 512                     # 16 supergroups of 512 tokens
SCALE = 1.0 / 8.0                   # 1/sqrt(HS)
WQK_S = 32.0                        # fp8 range scale on Wq/Wk
WO_S = 16.0                         # scale on Wo (cancels v's 1/16)
PM = 144                            # padded DoubleRow stationary stride
PN = 528                            # padded DoubleRow moving stride
WQK_COLS = 4 * 2 * 2 * PM           # 2304
WVO_COLS = 2 * 2 * PN               # 2112
WALL_COLS = 2 * WQK_COLS + 2 * WVO_COLS  # 8832

_CACHE = {}


def _build(n_cores, with_bq, with_bk, with_bv, with_bo, ng=NG):
    import concourse.bacc as bacc
    import concourse.mybir as mybir
    import concourse.tile as tile

    f32 = mybir.dt.float32
    f32r = mybir.dt.float32r
    bf16 = mybir.dt.bfloat16
    fp16 = mybir.dt.float16
    fp8 = mybir.dt.float8e4
    AFT = mybir.ActivationFunctionType
    DR = mybir.MatmulPerfMode.DoubleRow
    SUB = mybir.AluOpType.subtract

    tok = ng * 512

    nc = bacc.Bacc("TRN2", target_bir_lowering=False, debug=False,
                   num_devices=n_cores)

    def copy_op(eng, dst, src):
        if eng is nc.scalar:
            eng.copy(dst, src)
        else:
            eng.tensor_copy(dst, src)

    # x8: [p, z(hi/lo), c(4), t] merged fp8 residual pair, feature-major
    x8_d = nc.dram_tensor("x8", [128, 2 * 4 * tok], fp8,
                          kind="ExternalInput").ap()
    wall_d = nc.dram_tensor("wall", [128, WALL_COLS], fp8,
                            kind="ExternalInput").ap()
    wob_d = nc.dram_tensor("wob", [128, 4 * 512], fp16,
                           kind="ExternalInput").ap()
    bias_d = {}
    for name, used, dt_b in (("bq", with_bq, f32), ("bk", with_bk, f32),
                             ("bv", with_bv, bf16), ("bo", with_bo, f32r)):
        if used:
            bias_d[name] = nc.dram_tensor(name, [D], dt_b,
                                          kind="ExternalInput").ap()
    # DRAM bounce for the v -> vpad partition shuffle (double phase)
    vdr_d = nc.dram_tensor("vdr", [2 * 512, 512], fp16, kind="Internal").ap()
    y_d = nc.dram_tensor("y", [tok, D], fp16, kind="ExternalOutput").ap()

    with tile.TileContext(nc) as tc:
        with (
            tc.tile_pool(name="const", bufs=1) as cpool,
            tc.tile_pool(name="work", bufs=2) as wpool,
            tc.tile_pool(name="psum", bufs=5, space="PSUM") as ppool,
            tc.tile_pool(name="attn", bufs=3, space="PSUM") as apool,
        ):
            # ---- constants: three DMAs for all weights (q/k part first
            # so the opening projections start sooner) ----
            wall_sb = cpool.tile([128, WALL_COLS], fp8, name="wall_sb")
            nqk = 2 * WQK_COLS
            nc.scalar.dma_start(out=wall_sb[:, 0:WQK_COLS],
                                in_=wall_d[:, 0:WQK_COLS])
            nc.scalar.dma_start(out=wall_sb[:, WQK_COLS:nqk],
                                in_=wall_d[:, WQK_COLS:nqk])
            nc.scalar.dma_start(out=wall_sb[:, nqk:WALL_COLS],
                                in_=wall_d[:, nqk:WALL_COLS])
            wob_sb = cpool.tile([128, 4 * 512], fp16, name="wob_sb")
            nc.scalar.dma_start(out=wob_sb[:], in_=wob_d[:])
            woff = {}
            off = 0
            for name in ("wqh", "wkh"):
                woff[name] = off
                off += WQK_COLS
            for name in ("wvh", "wvl"):
                woff[name] = off
                off += WVO_COLS

            bias_sb = {}
            for name, ap_d in bias_d.items():
                if name not in ("bq", "bk"):
                    continue
                b_t = cpool.tile([128, 4], f32, name=f"{name}_sb")
                nc.scalar.dma_start(
                    out=b_t[:],
                    in_=ap_d.rearrange("(c p) -> p c", p=128))
                bias_sb[name] = b_t
            ones_sb = ones_bf_sb = None
            if with_bo:
                ones_sb = cpool.tile([1, 128], f32r, name="ones_sb")
                nc.gpsimd.memset(ones_sb[:], 1.0)
            if with_bv:
                ones_bf_sb = cpool.tile([1, 128], bf16, name="ones_bf_sb")
                nc.gpsimd.memset(ones_bf_sb[:], 1.0)
            bvrow_sb = bohrow_sb = None
            if with_bv:
                bvrow_sb = cpool.tile([1, 512], bf16, name="bvrow_sb")
                nc.scalar.dma_start(out=bvrow_sb[:],
                                    in_=bias_d["bv"].unsqueeze(0))
            if with_bo:
                bohrow_sb = cpool.tile([1, 512], f32r, name="bohrow_sb")
                nc.scalar.dma_start(out=bohrow_sb[:],
                                    in_=bias_d["bo"].unsqueeze(0))

            # kslab[c]: [128 feat, (z, 32w, 2j, 16s)] bf16 -- stationary for
            # packed scores; col (j,s) is nonzero on rows j*64..j*64+63 only,
            # the other half is zeroed once here and never overwritten.
            kslab = [cpool.tile([128, 2 * 1024], fp16, name=f"ks{c}")
                     for c in range(4)]
            for c in range(4):
                v = kslab[c].rearrange("p (z w j s) -> p z w j s",
                                       z=2, w=32, j=2, s=16)
                nc.gpsimd.memset(v[0:64, :, :, 1, :], 0.0)
                nc.gpsimd.memset(v[64:128, :, :, 0, :], 0.0)
            # vpad_lo/hi: [64, (z, 8wl, 2q, 4g, 64d)] bf16 -- ctx stationary
            # bands at rows (2m+q)%4*16+s.  Each 32-row band is 2x2 block-
            # diagonal in (row-half, q): the off-diagonal quadrants must be
            # zero; they are zeroed once here and never overwritten.
            vpad = [cpool.tile([64, 2 * 4096], fp16, name=f"vp{j}")
                    for j in range(2)]
            # engine ops cannot start at partition 16, so zero-fill the
            # quadrants by DMA from a zeroed staging tile; same sync queue
            # as the band-gather writers keeps ordering trivially correct.
            zpad = cpool.tile([16, 4096], fp16, name="zpad")
            nc.vector.memset(zpad[:], 0.0)

            def emit_vpad_zero_fills(z):
                zsrc = zpad.rearrange("p (a b) -> p a b", b=64)
                for j in range(2):
                    for r in range(4):
                        dstz = vpad[j].rearrange(
                            "(r s) (z gwl q d) -> r s z gwl q d",
                            s=16, z=2, gwl=32, q=2, d=64)[
                                r, :, z, :, 1 - (r % 2)]
                        nc.sync.dma_start(out=dstz, in_=zsrc[:, 0:32])

            def w_qk_slice(name, c, P):
                v = wall_sb[:, woff[name]:woff[name] + WQK_COLS].rearrange(
                    "p (c P t m) -> p c P t m", c=4, P=2, m=PM)
                return v[:, c, P, :, 0:128]

            def wo_slice(name, Q):
                v = wall_sb[:, woff[name]:woff[name] + WVO_COLS].rearrange(
                    "p (Q t n) -> p Q t n", Q=2, n=PN)
                return v[:, Q, :, 0:512]

            # ---- per-supergroup emitters (2-stage software pipeline) ----
            def load_x(G):
                """One DMA for the merged hi/lo feature-major x slice."""
                t8 = wpool.tile([128, 8 * PN], fp8, name="x8t", tag="x8t")
                nc.sync.dma_start(
                    out=t8.rearrange("p (zc n) -> p zc n", n=PN)[:, :, 0:512],
                    in_=x8_d.rearrange("p (zc t) -> p zc t", zc=8)[
                        :, :, G * 512:(G + 1) * 512])
                return t8

            def x_pair(t8, z, P):
                v = t8.rearrange("p (z P t n) -> p z P t n", z=2, P=2, n=PN)
                return v[:, z, P, :, 0:512]

            def proj_qk_mms(t8, wname, pj_ps):
                # 2-term residual: full-x (hi+lo) against W-hi only.  The
                # dropped x*Wl term costs ~1e-2 relative error end-to-end
                # (sigmoid damps the scores path), well under the 2e-2 gate.
                for P in range(2):
                    for z in range(2):
                        nc.tensor.matmul(
                            pj_ps[:], w_qk_slice(f"{wname}h", c_cur[0], P),
                            x_pair(t8, z, P),
                            start=(P == 0 and z == 0),
                            stop=(P == 1 and z == 1),
                            perf_mode=DR)

            c_cur = [0]

            def proj_q_chunk(t8, dst, c):
                c_cur[0] = c
                pj_ps = ppool.tile([128, 512], f32, name="pj_ps", tag="ps")
                proj_qk_mms(t8, "wq", pj_ps)
                if "bq" in bias_sb:
                    nc.scalar.activation(
                        dst[c][:], pj_ps[:], AFT.Identity,
                        bias=bias_sb["bq"][:, c:c + 1], scale=1.0 / 1024.0)
                else:
                    nc.vector.tensor_scalar_mul(dst[c][:], pj_ps[:],
                                                1.0 / 1024.0)

            def proj_k_chunk(t8, c, ph):
                """k^T chunk -> half-zero kslab stationary blocks."""
                c_cur[0] = c
                pj_ps = ppool.tile([128, 512], f32, name="pj_ps", tag="ps")
                proj_qk_mms(t8, "wk", pj_ps)
                dst = kslab[c].rearrange("p (z w j s) -> p z w j s",
                                         z=2, w=32, j=2, s=16)[:, ph]
                src = pj_ps.rearrange("p (w s) -> p w s", s=16)
                if "bk" in bias_sb:
                    nc.scalar.activation(
                        dst[0:64, :, 0, :], src[0:64], AFT.Identity,
                        bias=bias_sb["bk"][0:64, c:c + 1])
                    nc.scalar.activation(
                        dst[64:128, :, 1, :], src[64:128],
                        AFT.Identity, bias=bias_sb["bk"][64:128, c:c + 1])
                else:
                    # one engine per kslab tile (dep-tracker constraint)
                    eng = (nc.vector, nc.scalar, nc.vector, nc.scalar)[c]
                    copy_op(eng, dst[0:64, :, 0, :], src[0:64])
                    copy_op(eng, dst[64:128, :, 1, :], src[64:128])

            def proj_v(t8, ph):
                """v (token-major, bf16, 1/16-scaled) -> DRAM bounce ->
                vpad band gathers.  All on the sync queue (FIFO ordering
                substitutes for the untracked DRAM RAW dependency)."""
                v_sb = wpool.tile([128, 2048], fp16, name="v_sb", tag="v_sb")
                xsv = t8.rearrange("p (z P t n) -> p z P t n",
                                   z=2, P=2, n=PN)
                for g in range(4):
                    v_ps = ppool.tile([128, 512], f32, name="v_ps", tag="ps")
                    for P in range(2):
                        st_h = xsv[:, 0, P, :, g * 128:(g + 1) * 128]
                        st_l = xsv[:, 1, P, :, g * 128:(g + 1) * 128]
                        prods = ((st_h, "wvh"), (st_h, "wvl"), (st_l, "wvh"))
                        for i, (st, wn) in enumerate(prods):
                            nc.tensor.matmul(
                                v_ps[:], st, wo_slice(wn, P),
                                start=(P == 0 and i == 0),
                                stop=(P == 1 and i == 2 and not with_bv),
                                perf_mode=DR)
                    if with_bv:
                        nc.tensor.matmul(v_ps[:], ones_bf_sb[:], bvrow_sb[:],
                                         start=False, stop=True)
                    # v_sb cols are (g, hp, q, d): plain contiguous copy
                    nc.scalar.activation(v_sb[:, g * 512:(g + 1) * 512],
                                         v_ps[:], AFT.Identity,
                                         scale=1.0 / 512.0)
                # bounce: vdr row = (ph, g, wl*16+s), col = (hp q d)
                vdrv = vdr_d.rearrange("(z g p) c -> z p g c", z=2, g=4)
                nc.sync.dma_start(
                    out=vdrv[ph],
                    in_=v_sb.rearrange("p (g c) -> p g c", g=4))
                # band gather: dims [s, (g wl), d] on both sides
                srcb = vdr_d.rearrange(
                    "(z gwl s) (hp q d) -> z hp q s gwl d",
                    s=16, z=2, gwl=32, hp=4, q=2, d=64)
                for m in range(4):
                    dstp = vpad[m // 2].rearrange(
                        "(r s) (z gwl q d) -> r z q s gwl d",
                        s=16, z=2, gwl=32, q=2, d=64)
                    for q in range(2):
                        nc.sync.dma_start(
                            out=dstp[2 * (m % 2) + q, ph, q],
                            in_=srcb[ph, m, q])

            def ks_win(c, ph, w):
                """[128, 32] stationary slice of kslab[c], phase ph, win w."""
                return kslab[c].rearrange("p (z w n) -> p z w n",
                                          z=2, n=32)[:, ph, w]

            def scores(ph, qt, sc2):
                """packed scores: 4 independent 16-col matmuls per window."""
                for w in range(32):
                    wl = w % 16
                    cols = slice(wl * 16, (wl + 1) * 16)
                    for c in range(4):
                        sc = sc2[(c // 2) * 2 + w // 16]
                        rows = slice((c % 2) * 32, (c % 2) * 32 + 32)
                        nc.tensor.matmul(
                            sc[rows, cols], ks_win(c, ph, w),
                            qt[c][:, w * 16:(w + 1) * 16],
                            start=True, stop=True)

            def ctx(ph, pb2):
                """packed ctx^T (32-contraction) + fp16 stationary copy."""
                cs = [wpool.tile([128, 2 * 4 * 128], fp16, name=f"cs{Q}",
                                 tag=f"cs{Q}") for Q in range(2)]
                cs4 = [t.rearrange("p (t g n) -> p t g n", t=2, g=4, n=128)
                       for t in cs]
                for m in (2, 3, 0, 1):
                    cx = apool.tile([128, 512], f32, name=f"cx{m}",
                                     tag="at")
                    vpv = vpad[m // 2].rearrange(
                        "r (z gwl qd) -> r z gwl qd", z=2, qd=128)
                    rlo = (m % 2) * 32
                    pb = pb2[m // 2]
                    for w in range(32):
                        nc.tensor.matmul(
                            cx[:, w * 16:(w + 1) * 16],
                            vpv[rlo:rlo + 32, ph, w],
                            pb[rlo:rlo + 32, w * 16:(w + 1) * 16],
                            start=True, stop=True)
                    Q, t = m // 2, m % 2
                    src = cx.rearrange("p (g n) -> p g n", g=4)
                    # one engine per cs tile (dep-tracker constraint);
                    # GPSIMD cannot read PSUM, so only ACT/DVE qualify.
                    eng = nc.scalar if Q == 0 else nc.vector
                    copy_op(eng, cs4[Q][:, t], src)
                return cs4

            def out_proj(cs4, y_base):
                ot = [wpool.tile([128, 1024], fp16, name=f"ot{j}",
                                 tag=f"ot{j}") for j in range(2)]
                for g in range(4):
                    o_ps = ppool.tile([128, 512], f32, name="o_ps", tag="ps")
                    for m in range(4):
                        nc.tensor.matmul(
                            o_ps[:], cs4[m // 2][:, m % 2, g, :],
                            wob_sb[:, m * 512:(m + 1) * 512],
                            start=(m == 0),
                            stop=(m == 3 and not with_bo))
                    if with_bo:
                        nc.tensor.matmul(o_ps[:], ones_sb[:], bohrow_sb[:],
                                         start=False, stop=True)
                    dst = ot[g // 2][:, (g % 2) * 512:(g % 2) * 512 + 512]
                    eng = nc.vector
                    copy_op(eng, dst, o_ps[:])
                for j in range(2):
                    nc.scalar.dma_start(
                        out=y_d.rearrange("(r p) n -> p r n", p=128)[
                            :, 2 * j + y_base // 128:
                            2 * j + 2 + y_base // 128, :],
                        in_=ot[j].rearrange("p (r n) -> p r n", r=2))

            # ---- pipelined emission ----
            # per iteration: scores(P) -> sigmoid(P) -> proj_qk(G) ->
            # ctx(P) -> proj_v(G)+vpad DMAs(G) -> out_proj(P).
            x_next = load_x(0)
            prev = None  # (ph, qt)
            for G in range(ng + 1):
                ph = G % 2
                if G < ng:
                    t8 = x_next
                if G < 2:
                    emit_vpad_zero_fills(G)
                if prev is not None:
                    # four half-width score tiles -> 8 allocations per
                    # iteration on the 4-buffer "at" tag, so scores always
                    # reuse buffers freed in the PREVIOUS iteration.
                    sc2 = [apool.tile([64, 256], f32, name=f"sc{i}",
                                       tag="at") for i in range(4)]
                    pb2 = [wpool.tile([64, 512], fp16, name=f"pb{i}",
                                      tag=f"pb{i}") for i in range(2)]
                    scores(prev[0], prev[1], sc2)
                    for i in range(4):
                        nc.scalar.activation(
                            pb2[i // 2][:, (i % 2) * 256:(i % 2) * 256 + 256],
                            sc2[i][:], AFT.Sigmoid)
                if prev is not None:
                    cs4 = ctx(prev[0], pb2)
                if G < ng:
                    qt = [wpool.tile([128, 512], fp16, name=f"wqt{c}",
                                     tag=f"wqt{c}") for c in range(4)]
                    for c in range(4):
                        proj_q_chunk(t8, qt, c)
                        proj_k_chunk(t8, c, ph)
                    proj_v(t8, ph)
                    if G + 1 < ng:
                        x_next = load_x(G + 1)
                if prev is not None:
                    out_proj(cs4, (G - 1) * 512)
                prev = (ph, qt) if G < ng else None

    nc.compile()
    return nc


def _get_nc(n_cores, flags):
    key = (n_cores, flags)
    if key not in _CACHE:
        _CACHE[key] = _build(n_cores, *flags)
    return _CACHE[key]


def _fp8_split(a, np8):
    hi = a.astype(np8)
    lo = (a - hi.astype(np.float32)).astype(np8)
    return hi, lo


def _pack_wqk(w, np8):
    """[512, 512] -> [128, (c4, P2, t2, PM)] DoubleRow stationary layout."""
    out = np.zeros((128, 4, 2, 2, PM), np.float32)
    for c in range(4):
        for P in range(2):
            for t in range(2):
                blk = w[(2 * P + t) * 128:(2 * P + t + 1) * 128,
                        c * 128:(c + 1) * 128]
                out[:, c, P, t, 0:128] = blk
    return _fp8_split(np.ascontiguousarray(out.reshape(128, -1)), np8)


def _pack_wo(w):
    """[512, 512] -> [128, (Q2, t2, PN)] DoubleRow moving layout."""
    out = np.zeros((128, 2, 2, PN), np.float32)
    for Q in range(2):
        for t in range(2):
            out[:, Q, t, 0:512] = w[(2 * Q + t) * 128:(2 * Q + t + 1) * 128, :]
    return np.ascontiguousarray(out.reshape(128, -1))


def host_prep(x, Wq, bq, Wk, bk, Wv, bv, Wo, bo, flags):
    """Build the per-core device input dicts."""
    import concourse.mybir as mybir

    np8 = mybir.dt.np(mybir.dt.float8e4)
    # feature-major (pre-transposed) x copies, merged [p, z, c, t]
    xT = np.ascontiguousarray(
        np.asarray(x, np.float32).reshape(N_CORES, TOK, D)
        .transpose(0, 2, 1))                       # [cores, D, TOK]
    xh = xT.astype(np8)
    xl = (xT - xh.astype(np.float32)).astype(np8)
    # [cores, (c p), t] -> [cores, p, z, c, t]
    x8 = np.stack([xh.reshape(N_CORES, 4, 128, TOK),
                   xl.reshape(N_CORES, 4, 128, TOK)], axis=1)
    x8 = np.ascontiguousarray(x8.transpose(0, 3, 1, 2, 4)  # c,z? ->p,z,c,t
                              .reshape(N_CORES, 128, 2 * 4 * TOK))

    wqh, _ = _pack_wqk(np.asarray(Wq, np.float32) * (SCALE * WQK_S), np8)
    wkh, _ = _pack_wqk(np.asarray(Wk, np.float32) * WQK_S, np8)
    wvh, wvl = _fp8_split(_pack_wo(np.asarray(Wv, np.float32) * WQK_S), np8)
    wall = np.concatenate([wqh, wkh, wvh, wvl], axis=1)
    wall = np.ascontiguousarray(wall)
    # fp16 Wo moving operand: [128, (m, 512)], rows grouped by 128-tile
    wob = np.ascontiguousarray(
        (np.asarray(Wo, np.float32) * WO_S).reshape(4, 128, 512)
        .transpose(1, 0, 2).reshape(128, 4 * 512).astype(np.float16))
    base = {"wall": wall, "wob": wob}
    for name, b, used, scale in (("bq", bq, flags[0], SCALE),
                                 ("bk", bk, flags[1], WQK_S),
                                 ("bv", bv, flags[2], 512.0 / 16.0),
                                 ("bo", bo, flags[3], 1.0)):
        if used:
            arr = np.asarray(b, np.float32) * scale
            if name == "bv":
                arr = arr.astype(ml_dtypes.bfloat16)
            base[name] = np.ascontiguousarray(arr)

    return [dict(base, x8=x8[i]) for i in range(N_CORES)]


def kernel(x, Wq, bq, Wk, bk, Wv, bv, Wo, bo):
    from concourse.bass_utils import run_bass_kernel_spmd

    in_dt = x.dtype
    flags = tuple(bool(np.any(b)) for b in (bq, bk, bv, bo))
    nc = _get_nc(N_CORES, flags)
    in_maps = host_prep(x, Wq, bq, Wk, bk, Wv, bv, Wo, bo, flags)
    res = run_bass_kernel_spmd(nc, in_maps, list(range(N_CORES)))
    out = np.concatenate([res.results[i]["y"] for i in range(N_CORES)], axis=0)
    return (out.reshape(B, NH, NW, T, D).astype(np.float32)
            .astype(in_dt, copy=False))


# revision 66
# speedup vs baseline: 1.0320x; 1.0140x over previous
"""Trainium2 Bass kernel for windowed sigmoid-attention (nn_Attention_24927990186215).

Reference computation (per full input):
    x: [16, 16, 16, 16, 512]  (b, nh, nw, t, d) -- windows of T=16 tokens
    q/k/v = x @ W{q,k,v} + b{q,k,v}; split into H=8 heads of 64
    scores = q @ k^T / sqrt(64) within each 16-token window
    probs = sigmoid(scores)  (elementwise, NOT softmax)
    ctx = probs @ v;  out = ctx @ Wo + bo
Sharding: data-parallel over batch dim (16) across 8 cores -> 2 batches
(8192 tokens) per core.

Per-core dataflow (all matmuls on the PE):
  - host ships a merged feature-major fp8 hi/lo residual copy of x
    (x ~ xh + xl) plus one concatenated DoubleRow-packed fp8 hi/lo
    weight blob.  q^T/k^T/v are computed with fp8 DoubleRow matmuls
    (three residual cross terms; the xl*Wl term is dropped, ~0.1% err).
  - scores exploit the block-diagonal (windowed) structure: each window's
    [8 heads x 16 x 16] score block is computed PACKED, partition=(h,s),
    cols=t.  Heads 0-3 land in one [64,512] psum tile, heads 4-7 in a
    second, so each chunk's band starts at partition 0 or 32 (the hw only
    allows matmul output bases 0/32/64).  4 independent bf16 matmuls of
    16 output columns per window against half-zero "kslab" stationaries
    built from k^T.  2x fewer PE columns than dense [128,128] blocks,
    sigmoid runs once per [64,512] tile (8x less activation work), no
    mask multiply at all.
  - v bounces through DRAM (SBUF partition-split DMA sources are
    unreliable) and is gathered into "vpad" stationary tiles
    [32 (h,s) rows, (q,d) cols] -- pure data, no zero padding -- so
    ctx^T is computed packed too: one [32-contraction, 16-col] matmul
    per (head-pair, window) accumulating feature-major ctx^T tiles
    [128 (h,d), 512 tokens].  The DRAM bounce write and its dependent
    gathers share one engine queue (FIFO) because DRAM RAW deps are not
    tracked by the tile framework.
  - ctx^T converts to fp16 stationary tiles and the output projection
    runs as 4 chained bf16-rate matmuls per 128-token group against an
    fp16 Wo; y is staged in fp16 and written with two DMAs per 512
    tokens.  Intermediates use fp16 (not bf16) for error headroom: the
    2-term q/k projections land ~1.4e-2 relative error end-to-end.
  - dependency-tracker constraint honored throughout: all repeated
    writers of one SBUF tile use a single engine (readers only inherit
    the last writing engine per region).

Biases are folded in only when nonzero (the spec fills them with zeros).
"""

import numpy as np
import ml_dtypes

# ---- problem constants (hardcoded per the task contract) ----
N_CORES = 8
B, NH, NW, T, D = 16, 16, 16, 16, 512
HEADS, HS = 8, 64
TOK = (B // N_CORES) * NH * NW * T  # 8192 tokens per core
NG = TOK // 512                     # 16 supergroups of 512 tokens
SCALE = 1.0 / 8.0                   # 1/sqrt(HS)
WQK_S = 32.0                        # fp8 range scale on Wq/Wk
WO_S = 16.0                         # scale on Wo (cancels v's 1/16)
PM = 144                            # padded DoubleRow stationary stride
PN = 528                            # padded DoubleRow moving stride
WQK_COLS = 4 * 2 * 2 * PM           # 2304
WVO_COLS = 2 * 2 * PN               # 2112
WALL_COLS = 2 * WQK_COLS + 2 * WVO_COLS  # 8832

_CACHE = {}


def _build(n_cores, with_bq, with_bk, with_bv, with_bo, ng=NG):
    import concourse.bacc as bacc
    import concourse.mybir as mybir
    import concourse.tile as tile

    f32 = mybir.dt.float32
    f32r = mybir.dt.float32r
    bf16 = mybir.dt.bfloat16
    fp16 = mybir.dt.float16
    fp8 = mybir.dt.float8e4
    AFT = mybir.ActivationFunctionType
    DR = mybir.MatmulPerfMode.DoubleRow
    SUB = mybir.AluOpType.subtract

    tok = ng * 512

    nc = bacc.Bacc("TRN2", target_bir_lowering=False, debug=False,
                   num_devices=n_cores)

    def copy_op(eng, dst, src):
        if eng is nc.scalar:
            eng.copy(dst, src)
        else:
            eng.tensor_copy(dst, src)

    # x8: [p, z(hi/lo), c(4), t] merged fp8 residual pair, feature-major
    x8_d = nc.dram_tensor("x8", [128, 2 * 4 * tok], fp8,
                          kind="ExternalInput").ap()
    wall_d = nc.dram_tensor("wall", [128, WALL_COLS], fp8,
                            kind="ExternalInput").ap()
    wob_d = nc.dram_tensor("wob", [128, 4 * 512], fp16,
                           kind="ExternalInput").ap()
    bias_d = {}
    for name, used, dt_b in (("bq", with_bq, f32), ("bk", with_bk, f32),
                             ("bv", with_bv, bf16), ("bo", with_bo, f32r)):
        if used:
            bias_d[name] = nc.dram_tensor(name, [D], dt_b,
                                          kind="ExternalInput").ap()
    # DRAM bounce for the v -> vpad partition shuffle (double phase)
    vdr_d = nc.dram_tensor("vdr", [2 * 512, 512], fp16, kind="Internal").ap()
    y_d = nc.dram_tensor("y", [tok, D], fp16, kind="ExternalOutput").ap()

    with tile.TileContext(nc) as tc:
        with (
            tc.tile_pool(name="const", bufs=1) as cpool,
            tc.tile_pool(name="work", bufs=2) as wpool,
            tc.tile_pool(name="psum", bufs=5, space="PSUM") as ppool,
            tc.tile_pool(name="attn", bufs=3, space="PSUM") as apool,
        ):
            # ---- constants: three DMAs for all weights (q/k part first
            # so the opening projections start sooner) ----
            wall_sb = cpool.tile([128, WALL_COLS], fp8, name="wall_sb")
            nqk = 2 * WQK_COLS
            nc.scalar.dma_start(out=wall_sb[:, 0:WQK_COLS],
                                in_=wall_d[:, 0:WQK_COLS])
            nc.scalar.dma_start(out=wall_sb[:, WQK_COLS:nqk],
                                in_=wall_d[:, WQK_COLS:nqk])
            nc.scalar.dma_start(out=wall_sb[:, nqk:WALL_COLS],
                                in_=wall_d[:, nqk:WALL_COLS])
            wob_sb = cpool.tile([128, 4 * 512], fp16, name="wob_sb")
            nc.scalar.dma_start(out=wob_sb[:], in_=wob_d[:])
            woff = {}
            off = 0
            for name in ("wqh", "wkh"):
                woff[name] = off
                off += WQK_COLS
            for name in ("wvh", "wvl"):
                woff[name] = off
                off += WVO_COLS

            bias_sb = {}
            for name, ap_d in bias_d.items():
                if name not in ("bq", "bk"):
                    continue
                b_t = cpool.tile([128, 4], f32, name=f"{name}_sb")
                nc.scalar.dma_start(
                    out=b_t[:],
                    in_=ap_d.rearrange("(c p) -> p c", p=128))
                bias_sb[name] = b_t
            ones_sb = ones_bf_sb = None
            if with_bo:
                ones_sb = cpool.tile([1, 128], f32r, name="ones_sb")
                nc.gpsimd.memset(ones_sb[:], 1.0)
            if with_bv:
                ones_bf_sb = cpool.tile([1, 128], bf16, name="ones_bf_sb")
                nc.gpsimd.memset(ones_bf_sb[:], 1.0)
            bvrow_sb = bohrow_sb = None
            if with_bv:
                bvrow_sb = cpool.tile([1, 512], bf16, name="bvrow_sb")
                nc.scalar.dma_start(out=bvrow_sb[:],
                                    in_=bias_d["bv"].unsqueeze(0))
            if with_bo:
                bohrow_sb = cpool.tile([1, 512], f32r, name="bohrow_sb")
                nc.scalar.dma_start(out=bohrow_sb[:],
                                    in_=bias_d["bo"].unsqueeze(0))

            # kslab[c]: [128 feat, (z, 32w, 2j, 16s)] bf16 -- stationary for
            # packed scores; col (j,s) is nonzero on rows j*64..j*64+63 only,
            # the other half is zeroed once here and never overwritten.
            kslab = [cpool.tile([128, 2 * 1024], fp16, name=f"ks{c}")
                     for c in range(4)]
            for c in range(4):
                v = kslab[c].rearrange("p (z w j s) -> p z w j s",
                                       z=2, w=32, j=2, s=16)
                nc.gpsimd.memset(v[0:64, :, :, 1, :], 0.0)
                nc.gpsimd.memset(v[64:128, :, :, 0, :], 0.0)
            # vpad_lo/hi: [64, (z, 8wl, 2q, 4g, 64d)] bf16 -- ctx stationary
            # bands at rows (2m+q)%4*16+s.  Each 32-row band is 2x2 block-
            # diagonal in (row-half, q): the off-diagonal quadrants must be
            # zero; they are zeroed once here and never overwritten.
            vpad = [cpool.tile([64, 2 * 4096], fp16, name=f"vp{j}")
                    for j in range(2)]
            # engine ops cannot start at partition 16, so zero-fill the
            # quadrants by DMA from a zeroed staging tile; same sync queue
            # as the band-gather writers keeps ordering trivially correct.
            zpad = cpool.tile([16, 4096], fp16, name="zpad")
            nc.vector.memset(zpad[:], 0.0)

            def emit_vpad_zero_fills(z):
                zsrc = zpad.rearrange("p (a b) -> p a b", b=64)
                for j in range(2):
                    for r in range(4):
                        dstz = vpad[j].rearrange(
                            "(r s) (z gwl q d) -> r s z gwl q d",
                            s=16, z=2, gwl=32, q=2, d=64)[
                                r, :, z, :, 1 - (r % 2)]
                        nc.sync.dma_start(out=dstz, in_=zsrc[:, 0:32])

            def w_qk_slice(name, c, P):
                v = wall_sb[:, woff[name]:woff[name] + WQK_COLS].rearrange(
                    "p (c P t m) -> p c P t m", c=4, P=2, m=PM)
                return v[:, c, P, :, 0:128]

            def wo_slice(name, Q):
                v = wall_sb[:, woff[name]:woff[name] + WVO_COLS].rearrange(
                    "p (Q t n) -> p Q t n", Q=2, n=PN)
                return v[:, Q, :, 0:512]

            # ---- per-supergroup emitters (2-stage software pipeline) ----
            def load_x(G):
                """One DMA for the merged hi/lo feature-major x slice."""
                t8 = wpool.tile([128, 8 * PN], fp8, name="x8t", tag="x8t")
                nc.sync.dma_start(
                    out=t8.rearrange("p (zc n) -> p zc n", n=PN)[:, :, 0:512],
                    in_=x8_d.rearrange("p (zc t) -> p zc t", zc=8)[
                        :, :, G * 512:(G + 1) * 512])
                return t8

            def x_pair(t8, z, P):
                v = t8.rearrange("p (z P t n) -> p z P t n", z=2, P=2, n=PN)
                return v[:, z, P, :, 0:512]

            def proj_qk_mms(t8, wname, pj_ps):
                # 2-term residual: full-x (hi+lo) against W-hi only.  The
                # dropped x*Wl term costs ~1e-2 relative error end-to-end
                # (sigmoid damps the scores path), well under the 2e-2 gate.
                for P in range(2):
                    for z in range(2):
                        nc.tensor.matmul(
                            pj_ps[:], w_qk_slice(f"{wname}h", c_cur[0], P),
                            x_pair(t8, z, P),
                            start=(P == 0 and z == 0),
                            stop=(P == 1 and z == 1),
                            perf_mode=DR)

            c_cur = [0]

            def proj_q_chunk(t8, dst, c):
                c_cur[0] = c
                pj_ps = ppool.tile([128, 512], f32, name="pj_ps", tag="ps")
                proj_qk_mms(t8, "wq", pj_ps)
                if "bq" in bias_sb:
                    nc.scalar.activation(
                        dst[c][:], pj_ps[:], AFT.Identity,
                        bias=bias_sb["bq"][:, c:c + 1], scale=1.0 / 1024.0)
                else:
                    nc.vector.tensor_scalar_mul(dst[c][:], pj_ps[:],
                                                1.0 / 1024.0)

            def proj_k_chunk(t8, c, ph):
                """k^T chunk -> half-zero kslab stationary blocks."""
                c_cur[0] = c
                pj_ps = ppool.tile([128, 512], f32, name="pj_ps", tag="ps")
                proj_qk_mms(t8, "wk", pj_ps)
                dst = kslab[c].rearrange("p (z w j s) -> p z w j s",
                                         z=2, w=32, j=2, s=16)[:, ph]
                src = pj_ps.rearrange("p (w s) -> p w s", s=16)
                if "bk" in bias_sb:
                    nc.scalar.activation(
                        dst[0:64, :, 0, :], src[0:64], AFT.Identity,
                        bias=bias_sb["bk"][0:64, c:c + 1])
                    nc.scalar.activation(
                        dst[64:128, :, 1, :], src[64:128],
                        AFT.Identity, bias=bias_sb["bk"][64:128, c:c + 1])
                else:
                    # one engine per kslab tile (dep-tracker constraint)
                    eng = (nc.vector, nc.scalar, nc.vector, nc.scalar)[c]
                    copy_op(eng, dst[0:64, :, 0, :], src[0:64])
                    copy_op(eng, dst[64:128, :, 1, :], src[64:128])

            def proj_v(t8, ph):
                """v (token-major, bf16, 1/16-scaled) -> DRAM bounce ->
                vpad band gathers.  All on the sync queue (FIFO ordering
                substitutes for the untracked DRAM RAW dependency)."""
                v_sb = wpool.tile([128, 2048], fp16, name="v_sb", tag="v_sb")
                xsv = t8.rearrange("p (z P t n) -> p z P t n",
                                   z=2, P=2, n=PN)
                for g in range(4):
                    v_ps = ppool.tile([128, 512], f32, name="v_ps", tag="ps")
                    for P in range(2):
                        st_h = xsv[:, 0, P, :, g * 128:(g + 1) * 128]
                        st_l = xsv[:, 1, P, :, g * 128:(g + 1) * 128]
                        prods = ((st_h, "wvh"), (st_h, "wvl"), (st_l, "wvh"))
                        for i, (st, wn) in enumerate(prods):
                            nc.tensor.matmul(
                                v_ps[:], st, wo_slice(wn, P),
                                start=(P == 0 and i == 0),
                                stop=(P == 1 and i == 2 and not with_bv),
                                perf_mode=DR)
                    if with_bv:
                        nc.tensor.matmul(v_ps[:], ones_bf_sb[:], bvrow_sb[:],
                                         start=False, stop=True)
                    # v_sb cols are (g, hp, q, d): plain contiguous copy
                    nc.scalar.activation(v_sb[:, g * 512:(g + 1) * 512],
                                         v_ps[:], AFT.Identity,
                                         scale=1.0 / 512.0)
                # bounce: vdr row = (ph, g, wl*16+s), col = (hp q d)
                vdrv = vdr_d.rearrange("(z g p) c -> z p g c", z=2, g=4)
                nc.sync.dma_start(
                    out=vdrv[ph],
                    in_=v_sb.rearrange("p (g c) -> p g c", g=4))
                # band gather: dims [s, (g wl), d] on both sides
                srcb = vdr_d.rearrange(
                    "(z gwl s) (hp q d) -> z hp q s gwl d",
                    s=16, z=2, gwl=32, hp=4, q=2, d=64)
                for m in range(4):
                    dstp = vpad[m // 2].rearrange(
                        "(r s) (z gwl q d) -> r z q s gwl d",
                        s=16, z=2, gwl=32, q=2, d=64)
                    for q in range(2):
                        nc.sync.dma_start(
                            out=dstp[2 * (m % 2) + q, ph, q],
                            in_=srcb[ph, m, q])

            def ks_win(c, ph, w):
                """[128, 32] stationary slice of kslab[c], phase ph, win w."""
                return kslab[c].rearrange("p (z w n) -> p z w n",
                                          z=2, n=32)[:, ph, w]

            def scores(ph, qt, sc2):
                """packed scores: 4 independent 16-col matmuls per window."""
                for w in range(32):
                    wl = w % 16
                    cols = slice(wl * 16, (wl + 1) * 16)
                    for c in range(4):
                        sc = sc2[(c // 2) * 2 + w // 16]
                        rows = slice((c % 2) * 32, (c % 2) * 32 + 32)
                        nc.tensor.matmul(
                            sc[rows, cols], ks_win(c, ph, w),
                            qt[c][:, w * 16:(w + 1) * 16],
                            start=True, stop=True)

            def ctx(ph, pb2):
                """packed ctx^T (32-contraction) + fp16 stationary copy."""
                cs = [wpool.tile([128, 2 * 4 * 128], fp16, name=f"cs{Q}",
                                 tag=f"cs{Q}") for Q in range(2)]
                cs4 = [t.rearrange("p (t g n) -> p t g n", t=2, g=4, n=128)
                       for t in cs]
                for m in (2, 3, 0, 1):
                    cx = apool.tile([128, 512], f32, name=f"cx{m}",
                                     tag="at")
                    vpv = vpad[m // 2].rearrange(
                        "r (z gwl qd) -> r z gwl qd", z=2, qd=128)
                    rlo = (m % 2) * 32
                    pb = pb2[m // 2]
                    for w in range(32):
                        nc.tensor.matmul(
                            cx[:, w * 16:(w + 1) * 16],
                            vpv[rlo:rlo + 32, ph, w],
                            pb[rlo:rlo + 32, w * 16:(w + 1) * 16],
                            start=True, stop=True)
                    Q, t = m // 2, m % 2
                    src = cx.rearrange("p (g n) -> p g n", g=4)
                    # one engine per cs tile (dep-tracker constraint);
                    # GPSIMD cannot read PSUM, so only ACT/DVE qualify.
                    eng = nc.scalar if Q == 0 else nc.vector
                    copy_op(eng, cs4[Q][:, t], src)
                return cs4

            def out_proj(cs4, y_base):
                ot = [wpool.tile([128, 1024], fp16, name=f"ot{j}",
                                 tag=f"ot{j}") for j in range(2)]
                for g in range(4):
                    o_ps = ppool.tile([128, 512], f32, name="o_ps", tag="ps")
                    for m in range(4):
                        nc.tensor.matmul(
                            o_ps[:], cs4[m // 2][:, m % 2, g, :],
                            wob_sb[:, m * 512:(m + 1) * 512],
                            start=(m == 0),
                            stop=(m == 3 and not with_bo))
                    if with_bo:
                        nc.tensor.matmul(o_ps[:], ones_sb[:], bohrow_sb[:],
                                         start=False, stop=True)
                    dst = ot[g // 2][:, (g % 2) * 512:(g % 2) * 512 + 512]
                    eng = nc.vector if g < 2 else nc.scalar
                    copy_op(eng, dst, o_ps[:])
                for j in range(2):
                    # y stores ride the Pool SWDGE queue: keeps the HWDGE
                    # path clear for the latency-sensitive vpad gathers.
                    nc.gpsimd.dma_start(
                        out=y_d.rearrange("(r p) n -> p r n", p=128)[
                            :, 2 * j + y_base // 128:
                            2 * j + 2 + y_base // 128, :],
                        in_=ot[j].rearrange("p (r n) -> p r n", r=2))

            # ---- pipelined emission ----
            # per iteration: scores(P) -> sigmoid(P) -> proj_qk(G) ->
            # ctx(P) -> proj_v(G)+vpad DMAs(G) -> out_proj(P).
            x_next = load_x(0)
            prev = None  # (ph, qt)
            for G in range(ng + 1):
                ph = G % 2
                if G < ng:
                    t8 = x_next
                if G < 2:
                    emit_vpad_zero_fills(G)
                if prev is not None:
                    # four half-width score tiles -> 8 allocations per
                    # iteration on the 4-buffer "at" tag, so scores always
                    # reuse buffers freed in the PREVIOUS iteration.
                    sc2 = [apool.tile([64, 256], f32, name=f"sc{i}",
                                       tag="at") for i in range(4)]
                    pb2 = [wpool.tile([64, 512], fp16, name=f"pb{i}",
                                      tag=f"pb{i}") for i in range(2)]
                    scores(prev[0], prev[1], sc2)
                    for i in range(4):
                        nc.scalar.activation(
                            pb2[i // 2][:, (i % 2) * 256:(i % 2) * 256 + 256],
                            sc2[i][:], AFT.Sigmoid)
                if prev is not None:
                    cs4 = ctx(prev[0], pb2)
                if G < ng:
                    qt = [wpool.tile([128, 512], fp16, name=f"wqt{c}",
                                     tag=f"wqt{c}") for c in range(4)]
                    for c in range(4):
                        proj_q_chunk(t8, qt, c)
                        proj_k_chunk(t8, c, ph)
                    proj_v(t8, ph)
                    if G + 1 < ng:
                        x_next = load_x(G + 1)
                if prev is not None:
                    out_proj(cs4, (G - 1) * 512)
                prev = (ph, qt) if G < ng else None

    nc.compile()
    return nc


def _get_nc(n_cores, flags):
    key = (n_cores, flags)
    if key not in _CACHE:
        _CACHE[key] = _build(n_cores, *flags)
    return _CACHE[key]


def _fp8_split(a, np8):
    hi = a.astype(np8)
    lo = (a - hi.astype(np.float32)).astype(np8)
    return hi, lo


def _pack_wqk(w, np8):
    """[512, 512] -> [128, (c4, P2, t2, PM)] DoubleRow stationary layout."""
    out = np.zeros((128, 4, 2, 2, PM), np.float32)
    for c in range(4):
        for P in range(2):
            for t in range(2):
                blk = w[(2 * P + t) * 128:(2 * P + t + 1) * 128,
                        c * 128:(c + 1) * 128]
                out[:, c, P, t, 0:128] = blk
    return _fp8_split(np.ascontiguousarray(out.reshape(128, -1)), np8)


def _pack_wo(w):
    """[512, 512] -> [128, (Q2, t2, PN)] DoubleRow moving layout."""
    out = np.zeros((128, 2, 2, PN), np.float32)
    for Q in range(2):
        for t in range(2):
            out[:, Q, t, 0:512] = w[(2 * Q + t) * 128:(2 * Q + t + 1) * 128, :]
    return np.ascontiguousarray(out.reshape(128, -1))


def host_prep(x, Wq, bq, Wk, bk, Wv, bv, Wo, bo, flags):
    """Build the per-core device input dicts."""
    import concourse.mybir as mybir

    np8 = mybir.dt.np(mybir.dt.float8e4)
    # feature-major (pre-transposed) x copies, merged [p, z, c, t]
    xT = np.ascontiguousarray(
        np.asarray(x, np.float32).reshape(N_CORES, TOK, D)
        .transpose(0, 2, 1))                       # [cores, D, TOK]
    xh = xT.astype(np8)
    xl = (xT - xh.astype(np.float32)).astype(np8)
    # [cores, (c p), t] -> [cores, p, z, c, t]
    x8 = np.stack([xh.reshape(N_CORES, 4, 128, TOK),
                   xl.reshape(N_CORES, 4, 128, TOK)], axis=1)
    x8 = np.ascontiguousarray(x8.transpose(0, 3, 1, 2, 4)  # c,z? ->p,z,c,t
                              .reshape(N_CORES, 128, 2 * 4 * TOK))

    wqh, _ = _pack_wqk(np.asarray(Wq, np.float32) * (SCALE * WQK_S), np8)
    wkh, _ = _pack_wqk(np.asarray(Wk, np.float32) * WQK_S, np8)
    wvh, wvl = _fp8_split(_pack_wo(np.asarray(Wv, np.float32) * WQK_S), np8)
    wall = np.concatenate([wqh, wkh, wvh, wvl], axis=1)
    wall = np.ascontiguousarray(wall)
    # fp16 Wo moving operand: [128, (m, 512)], rows grouped by 128-tile
    wob = np.ascontiguousarray(
        (np.asarray(Wo, np.float32) * WO_S).reshape(4, 128, 512)
        .transpose(1, 0, 2).reshape(128, 4 * 512).astype(np.float16))
    base = {"wall": wall, "wob": wob}
    for name, b, used, scale in (("bq", bq, flags[0], SCALE),
                                 ("bk", bk, flags[1], WQK_S),
                                 ("bv", bv, flags[2], 512.0 / 16.0),
                                 ("bo", bo, flags[3], 1.0)):
        if used:
            arr = np.asarray(b, np.float32) * scale
            if name == "bv":
                arr = arr.astype(ml_dtypes.bfloat16)
            base[name] = np.ascontiguousarray(arr)

    return [dict(base, x8=x8[i]) for i in range(N_CORES)]


def kernel(x, Wq, bq, Wk, bk, Wv, bv, Wo, bo):
    from concourse.bass_utils import run_bass_kernel_spmd

    in_dt = x.dtype
    flags = tuple(bool(np.any(b)) for b in (bq, bk, bv, bo))
    nc = _get_nc(N_CORES, flags)
    in_maps = host_prep(x, Wq, bq, Wk, bk, Wv, bv, Wo, bo, flags)
    res = run_bass_kernel_spmd(nc, in_maps, list(range(N_CORES)))
    out = np.concatenate([res.results[i]["y"] for i in range(N_CORES)], axis=0)
    return (out.reshape(B, NH, NW, T, D).astype(np.float32)
            .astype(in_dt, copy=False))
